# revision 12
# baseline (speedup 1.0000x reference)
"""Causal self-attention block (nn_CrossAttention) on 8 TRN2 NeuronCores.

Sharding: data-parallel over batch (B=2 -> 2 groups of 4 cores), tensor-parallel
over heads within a group (16 heads -> 4 heads/core, splitting Wq/Wk/Wv rows and
Wp columns). Each core computes a full [N, DIM] partial of the output projection
for its 4 heads; the host sums the 4 partials per batch and adds the bias.

Device-side layout ("transposed world", everything feature-major):
  xT   [C=1024, N=2048]    QT/KT/VT = W @ xT -> [d, n] with d on partitions
  V    = PE-transpose of VT blocks -> [l, d] per 128-block, packed per head
         with a 64-wide ones block ([V_h|ones] even heads, [ones|V_h] odd)
  S^T  = K_j @ Q^T chunks  -> [l, n] in PSUM (l = key block on partitions)
  P^T  = exp(SCALE*S^T) -> SBUF bf16, causal-masked by a 0/1 multiply
  O''  = [V_j|ones].T @ P^T accumulated in PSUM: O rows + row-sum rows
  out  = (O/s).T-pair @ WpT -> [n, e] partial, f32 to DRAM

No max-subtraction is needed in the softmax (logits*scale max ~8).
Attention runs in two n-half passes per head so the O'' accumulator is
2 PSUM banks and rotates (next head starts while previous normalizes).
"""

import numpy as np
import ml_dtypes

B = 2
N = 2048
DIM = 1024
H = 16
D = 64
SCALE = D ** -0.5
NCORES = 8
HPC = 4          # heads per core
FPC = HPC * D    # feature rows per core (256)

_BF = ml_dtypes.bfloat16

_built = None


def _split512(lo, hi):
    """Split [lo, hi) at multiples of 512 (PSUM bank boundaries)."""
    out = []
    p = lo
    while p < hi:
        q = min(hi, (p // 512 + 1) * 512)
        out.append((p, q))
        p = q
    return out


def _build(loop_k=None):
    """Build the (SPMD, data-only-sharded) Bass program. Same NEFF on all cores.

    loop_k: if set, wrap the whole compute body in a hardware For-loop that
    repeats it loop_k times (used only for timing-by-differencing in bench.py;
    the graded path uses loop_k=None).
    """
    import concourse.bass as bass
    import concourse.mybir as mybir
    import concourse.tile as tile
    from concourse import bacc
    from contextlib import ExitStack

    bf16 = mybir.dt.bfloat16
    f32 = mybir.dt.float32
    Exp = mybir.ActivationFunctionType.Exp

    nc = bacc.Bacc()
    xT_d = nc.dram_tensor("xT", [DIM, N], bf16, kind="ExternalInput")
    wqT_d = nc.dram_tensor("wqT", [DIM, FPC], bf16, kind="ExternalInput")
    wkT_d = nc.dram_tensor("wkT", [DIM, FPC], bf16, kind="ExternalInput")
    wvT_d = nc.dram_tensor("wvT", [DIM, FPC], bf16, kind="ExternalInput")
    wpT_d = nc.dram_tensor("wpT", [FPC, DIM], bf16, kind="ExternalInput")
    mask_d = nc.dram_tensor("mask01", [128, 128], bf16, kind="ExternalInput")
    ident_d = nc.dram_tensor("ident", [128, 128], bf16, kind="ExternalInput")
    out_d = nc.dram_tensor("out", [N, DIM], f32, kind="ExternalOutput")

    NB = N // 128      # 16 blocks of 128 along sequence
    KC = DIM // 128    # 8 contraction chunks

    with tile.TileContext(nc) as tc, ExitStack() as ctx:
        sing = ctx.enter_context(tc.tile_pool(name="sing", bufs=1))
        pspool = ctx.enter_context(tc.tile_pool(name="pspool", bufs=2, space="PSUM"))
        o2pool = ctx.enter_context(tc.tile_pool(name="o2pool", bufs=2, space="PSUM"))
        ptpool = ctx.enter_context(tc.tile_pool(name="ptpool", bufs=4))
        rcpool = ctx.enter_context(tc.tile_pool(name="rcpool", bufs=2))
        outpool = ctx.enter_context(tc.tile_pool(name="outpool", bufs=3))

        if loop_k is not None:
            ctx.enter_context(tc.For_i(0, loop_k, 1))

        xTs = sing.tile([128, KC, N], bf16)
        wqTs = sing.tile([128, KC, FPC], bf16)
        wkTs = sing.tile([128, KC, FPC], bf16)
        wvTs = sing.tile([128, KC, FPC], bf16)
        wpTs = sing.tile([128, 2, DIM], bf16)
        qTs = sing.tile([128, 2, N], bf16)
        kTs = sing.tile([128, 2, N], bf16)
        vTs = sing.tile([128, 2, N], bf16)
        # v2: per (l-block j, head h) a contiguous 128-col weight slot:
        # even h -> [V_h | ones], odd h -> [ones | V_h]  (so O lands on
        # partitions [64*(h%2), +64) and the row-sums on the other half)
        v2 = sing.tile([128, NB, HPC, 128], bf16)
        onorm = sing.tile([128, 2, N], bf16)
        maskS = sing.tile([128, 128], bf16)
        identS = sing.tile([128, 128], bf16)

        # ---- input DMAs (first-needed first) ----
        nc.sync.dma_start(out=wqTs, in_=wqT_d[:].rearrange("(a p) d -> p a d", p=128))
        nc.sync.dma_start(out=wkTs, in_=wkT_d[:].rearrange("(a p) d -> p a d", p=128))
        for a in range(KC):
            nc.sync.dma_start(out=xTs[:, a, :], in_=xT_d[128 * a:128 * (a + 1), :])
        nc.sync.dma_start(out=wvTs, in_=wvT_d[:].rearrange("(a p) d -> p a d", p=128))
        nc.sync.dma_start(out=identS, in_=ident_d[:, :])
        nc.sync.dma_start(out=maskS, in_=mask_d[:, :])
        nc.sync.dma_start(out=wpTs, in_=wpT_d[:].rearrange("(a p) d -> p a d", p=128))

        for h in range(HPC):
            ones_cols = slice(64, 128) if h % 2 == 0 else slice(0, 64)
            nc.vector.memset(v2[:, :, h, ones_cols], 1.0)

        # ---- Q/K/V projections (weight-stationary, transposed outputs) ----
        def proj_block(wt, dst, t):
            for cc in range(2):
                ps = pspool.tile([128, 1024], f32, tag="ps", name=f"qkv_ps")
                for half in range(2):
                    n0 = 1024 * cc + 512 * half
                    for k in range(KC):
                        nc.tensor.matmul(
                            ps[:, 512 * half:512 * half + 512],
                            lhsT=wt[:, k, 128 * t:128 * (t + 1)],
                            rhs=xTs[:, k, n0:n0 + 512],
                            start=(k == 0), stop=(k == KC - 1),
                        )
                nc.vector.tensor_copy(
                    out=dst[:, t, 1024 * cc:1024 * (cc + 1)], in_=ps[:, :]
                )

        def v_transpose_block(t):
            # vTs[:, t, :] rows are d-dims of heads (2t, 2t+1); transpose each
            # 128x128 l-block back to [l, d] and scatter into the two heads'
            # v2 slots ([V|ones] / [ones|V]).
            vj_all = v2[:, :, :, :]
            part_d = list(vj_all.ap)[0]
            for j in range(NB):
                vt_ps = pspool.tile([128, 128], bf16, tag="ps", name="vt_ps")
                nc.tensor.transpose(
                    vt_ps[:, :], vTs[:, t, 128 * j:128 * (j + 1)], identS[:, :]
                )
                # dst: head 2t cols 0:64 and head 2t+1 cols 64:128
                dst = bass.AP(
                    tensor=vj_all.tensor,
                    offset=vj_all.offset + j * HPC * 128 + 256 * t,
                    ap=[[part_d[0], part_d[1]], [192, 2], [1, 64]],
                )
                nc.vector.tensor_copy(out=dst, in_=vt_ps[:, :])

        # ---- interleaved: QKV block t, then attention for heads (2t, 2t+1) ----
        # The pair's heads run j-by-j adjacent so their K=64 S^T matmuls
        # (base partitions 0 / 64) row-pack concurrently in the PE array.
        for t in range(2):
            proj_block(wqTs, qTs, t)
            proj_block(wkTs, kTs, t)
            proj_block(wvTs, vTs, t)
            v_transpose_block(t)

            for q in range(2):
                nlo, nhi = 1024 * q, 1024 * (q + 1)
                o2 = {}
                for par in range(2):  # even/odd head of the pair
                    o2[par] = o2pool.tile(
                        [128, 1024], f32, tag="o2", name=f"o2_{t}_{q}_{par}"
                    )
                for j in range(8 * (q + 1)):
                    a0 = 128 * j
                    lo, hi = max(a0, nlo), nhi
                    rel = nlo
                    pieces = _split512(lo, hi)
                    st = {}
                    for par in range(2):
                        r = 64 * par
                        st[par] = pspool.tile([128, 1024], f32, tag="ps", name="st")
                        for p0, p1 in pieces:
                            nc.tensor.matmul(
                                st[par][:, p0 - rel:p1 - rel],
                                lhsT=kTs[r:r + 64, t, a0:a0 + 128],
                                rhs=qTs[r:r + 64, t, p0:p1],
                                start=True, stop=True,
                            )
                    pt = {}
                    for par in range(2):
                        h = 2 * t + par
                        pt[par] = ptpool.tile([128, 1024], bf16, tag="pt", name="pt")
                        nc.scalar.activation(
                            out=pt[par][:, lo - rel:hi - rel],
                            in_=st[par][:, lo - rel:hi - rel],
                            func=Exp, scale=SCALE,
                        )
                        if lo == a0:  # this chunk starts at the diagonal block
                            nc.vector.tensor_mul(
                                pt[par][:, a0 - rel:a0 - rel + 128],
                                pt[par][:, a0 - rel:a0 - rel + 128],
                                maskS,
                            )
                    for par in range(2):
                        h = 2 * t + par
                        vap = v2[:, j, h, :]
                        for p0, p1 in pieces:
                            bank = p0 // 512
                            nc.tensor.matmul(
                                o2[par][:, p0 - rel:p1 - rel],
                                lhsT=vap,
                                rhs=pt[par][:, p0 - rel:p1 - rel],
                                start=(j == 0),
                                stop=(j == min(4 * bank + 3, 8 * (q + 1) - 1)),
                            )
                for par in range(2):
                    r = 64 * par
                    sb = 64 - r
                    rc = rcpool.tile([128, 1024], f32, tag="rc", name="rc")
                    nc.vector.reciprocal(out=rc[sb:sb + 64, :], in_=o2[par][sb:sb + 64, :])
                    # move 1/s onto O's partitions (DMA shuffles partitions; DVE cannot)
                    nc.sync.dma_start(out=rc[r:r + 64, :], in_=rc[sb:sb + 64, :])
                    nc.vector.tensor_mul(
                        out=onorm[r:r + 64, t, nlo:nhi], in0=o2[par][r:r + 64, :],
                        in1=rc[r:r + 64, :],
                    )

        # ---- output projection: out[n_blk, e] = sum_pair O_pair.T @ WpT_pair ----
        for nb in range(NB):
            po = pspool.tile([128, 1024], f32, tag="ps", name="po")
            for half in range(2):
                for p in range(2):
                    nc.tensor.matmul(
                        po[:, 512 * half:512 * half + 512],
                        lhsT=onorm[:, p, 128 * nb:128 * (nb + 1)],
                        rhs=wpTs[:, p, 512 * half:512 * half + 512],
                        start=(p == 0), stop=(p == 1),
                    )
            ostage = outpool.tile([128, 1024], f32, tag="ostage", name="ostage")
            if nb % 2 == 0:
                nc.scalar.copy(out=ostage, in_=po)
            else:
                nc.vector.tensor_copy(out=ostage, in_=po)
            nc.sync.dma_start(out=out_d[128 * nb:128 * (nb + 1), :], in_=ostage)

    nc.finalize()
    return nc


def _get_nc():
    global _built
    if _built is None:
        _built = _build()
    return _built


def make_in_maps(x, Wq, Wk, Wv, Wp):
    # 0 where key>query (strictly-lower in [l, n] coords), else 1
    mask = np.where(
        np.arange(128)[:, None] > np.arange(128)[None, :], 0.0, 1.0
    ).astype(_BF)
    ident = np.eye(128, dtype=np.float32).astype(_BF)
    in_maps = []
    for c in range(NCORES):
        b, g = c // HPC, c % HPC
        rows = slice(FPC * g, FPC * (g + 1))
        in_maps.append({
            "xT": np.ascontiguousarray(x[b].T).astype(_BF),
            "wqT": np.ascontiguousarray(Wq[rows, :].T).astype(_BF),
            "wkT": np.ascontiguousarray(Wk[rows, :].T).astype(_BF),
            "wvT": np.ascontiguousarray(Wv[rows, :].T).astype(_BF),
            "wpT": np.ascontiguousarray(Wp[:, rows].T).astype(_BF),
            "mask01": mask,
            "ident": ident,
        })
    return in_maps


def run_sharded(x, Wq, Wk, Wv, Wp, bp, trace=False, **spmd_kwargs):
    from concourse.bass_utils import run_bass_kernel_spmd

    nc = _get_nc()
    in_maps = make_in_maps(x, Wq, Wk, Wv, Wp)
    res = run_bass_kernel_spmd(
        nc, in_maps, core_ids=list(range(NCORES)), trace=trace, **spmd_kwargs
    )
    parts = [r["out"] for r in res.results]
    out = np.zeros((B, N, DIM), np.float32)
    for b in range(B):
        acc = np.zeros((N, DIM), np.float32)
        for g in range(HPC):
            acc += parts[b * HPC + g]
        out[b] = acc + bp.astype(np.float32)[None, :]
    return out, res


def kernel(x, y, Wq, Wk, Wv, Wp, bp):
    x = np.asarray(x, np.float32)
    out, _ = run_sharded(
        x,
        np.asarray(Wq, np.float32), np.asarray(Wk, np.float32),
        np.asarray(Wv, np.float32), np.asarray(Wp, np.float32),
        np.asarray(bp, np.float32),
    )
    return out
